# revision 20
# baseline (speedup 1.0000x reference)
"""Block-sparse (DeepSpeed fixed-layout) self-attention on 8 Trainium2 cores.

Strategy
--------
Shard the 32 (batch, head) slices across 8 cores (4 each, pure data parallel).
For each slice, queries are processed in windows of 128 rows (4 key-blocks of
32). The union of active key blocks for a window is split into "chunk slots"
of up to 128 keys; slots are deduplicated across windows (the fixed layout's
global stripe makes most slots shared). Host-side numpy pre-gathers:
  QT  [S, 64, L]            query transposed (hd on partitions)
  KTg [S, 64, nch*128]      gathered+transposed key chunks
  Vg  [S, 128, nch, 65]     gathered value chunks with a ones column
On device, per window and chunk:
  S_T  = KT_chunk.T-matmul (scores arrive keys-on-partitions: no P transpose)
  P    = exp(scale * S_T)   on ACT, straight from PSUM
  mask: memset invalid (key-block, query-block) cells to zero
  O~  += P.T @ [V | 1]      accumulated in PSUM; col 64 = softmax denominator
then O = O~[:, :64] * (1 / O~[:, 64]) and DMA out. exp() needs no max
subtraction: scores are ~N(0,1) after scaling, far from fp32 overflow.
"""

import sys

sys.path.insert(0, "/opt/trn_rl_repo")

import numpy as np

N_CORES = 8
_KVER = "v19"  # bump on any codegen change: feeds the rtag config fingerprint

# dtype knobs: storage+matmul dtype for scores (QT/KTg) and probs (P/Vg)
S_DT_NAME = "bfloat16"
P_DT_NAME = "bfloat16"

_cache = {}


def _build_plan(rows, cols, nb, qw):
    """Per query-window chunk lists + deduplicated key-chunk slots.

    Returns (windows, slot_blocks):
      windows: list (one per window) of chunks (slot_id, n_blocks, valid)
               where valid[kb, j] says whether key-block kb of the chunk is
               attended by query-block j of the window.
      slot_blocks: slot_id -> list of key block ids stored in that slot.
    """
    from collections import Counter

    row_cnt = [Counter() for _ in range(nb)]
    for r, c in zip(rows.tolist(), cols.tolist()):
        row_cnt[int(r)][int(c)] += 1

    slots = {}
    slot_blocks = []
    windows = []
    for w0 in range(0, nb, qw):
        cnts = [row_cnt[w0 + j] for j in range(qw)]
        cols_set = sorted(set().union(*[set(c.keys()) for c in cnts]))
        entries = []  # (block, occurrence)
        for c in cols_set:
            m = max(cnt[c] for cnt in cnts)
            entries.extend((c, k) for k in range(m))
        # maximal equal-stride runs -> chunk boundaries shared across windows
        runs = []
        i, n = 0, len(entries)
        while i < n:
            if i + 1 < n:
                stride = entries[i + 1][0] - entries[i][0]
                j = i + 1
                while j + 1 < n and entries[j + 1][0] - entries[j][0] == stride:
                    j += 1
            else:
                stride, j = 1, i
            runs.append((i, j + 1, stride))
            i = j + 1
        wchunks = []
        for a, b, stride in runs:
            for t in range(a, b, qw):
                grp = entries[t : min(t + qw, b)]
                start = grp[0][0]
                key = (start, stride if len(grp) > 1 else 1)
                sid = slots.get(key)
                if sid is None:
                    sid = len(slot_blocks)
                    slots[key] = sid
                    slot_blocks.append([])
                if len(slot_blocks[sid]) < len(grp):
                    slot_blocks[sid] = [start + key[1] * u for u in range(len(grp))]
                valid = np.ones((len(grp), qw), dtype=bool)
                for kb, (c, k) in enumerate(grp):
                    for j in range(qw):
                        valid[kb, j] = k < cnts[j][c]
                wchunks.append((sid, len(grp), valid))
        # does any query row of this window have no valid key at all?
        anyvalid = np.zeros(qw, dtype=bool)
        for _sid, _n, valid in wchunks:
            anyvalid |= valid.any(axis=0)
        windows.append((wchunks, not anyvalid.all()))
    return windows, slot_blocks


def _zero_regions(valid, bs):
    """Invalid (key-block, query-block) cells as memset rectangles."""
    regs = []
    nkb, qw = valid.shape
    for kb in range(nkb):
        j = 0
        while j < qw:
            if not valid[kb, j]:
                j0 = j
                while j < qw and not valid[kb, j]:
                    j += 1
                regs.append((kb * bs, (kb + 1) * bs, j0 * bs, j * bs))
            else:
                j += 1
    return regs


def _group_plan(windows, qw_groups=4):
    """Group consecutive windows; extract chunks shared (all-valid) by every
    window of the group so their S_T/exp run once at group width."""
    groups = []
    for g0 in range(0, len(windows), qw_groups):
        ws = windows[g0 : g0 + qw_groups]
        sets = [
            {(sid, nblk) for sid, nblk, valid in w if valid.all()}
            for w, _g in ws
        ]
        shared = sorted(set.intersection(*sets)) if len(ws) == qw_groups else []
        shared_set = set(shared)
        owns = []
        for w, guard in ws:
            owns.append(
                (
                    [
                        (sid, nblk, valid)
                        for sid, nblk, valid in w
                        if (sid, nblk) not in shared_set or not valid.all()
                    ],
                    guard,
                )
            )
        groups.append((shared, owns))
    return groups


QUAD_PACK = 4


def _quads(owns):
    """Flatten a group's per-window own chunks, QUAD_PACK halves per tile."""
    halves = []
    for m, (own, _guard) in enumerate(owns):
        for sid, nblk, valid in own:
            halves.append((m, sid, nblk, valid))
    return [halves[i : i + QUAD_PACK] for i in range(0, len(halves), QUAD_PACK)]


def _quad_sig(quad):
    # mask pattern depends only on each half's (nblk, validity) - not sid
    return tuple((nblk, valid.tobytes()) for _m, _sid, nblk, valid in quad)


def _mask_table(windows, qw_groups=4):
    """Dedup mask patterns over own-chunk quads. Returns (n_masks, map from
    quad-signature -> mask id); quads with no zero region map to nothing."""
    mk_of = {}
    n = 0
    for shared, owns in _group_plan(windows, qw_groups):
        for quad in _quads(owns):
            sig = _quad_sig(quad)
            if sig in mk_of:
                continue
            if any(not valid.all() for _m, _sid, _nblk, valid in quad):
                mk_of[sig] = n
                n += 1
    return n, mk_of


def _build_masks(windows, dims, p_np, qw_groups=4):
    """Materialize the deduplicated mask tiles: [n_mk, 128, 4, nq]."""
    S, L, HD, bs, qw, nch = dims
    nq = qw * bs
    n_mk, mk_of = _mask_table(windows, qw_groups)
    mk = np.ones((max(n_mk, 1), 128, QUAD_PACK, nq), p_np)
    done = set()
    for shared, owns in _group_plan(windows, qw_groups):
        for quad in _quads(owns):
            mid = mk_of.get(_quad_sig(quad))
            if mid is None or mid in done:
                continue
            done.add(mid)
            for h, (_m, _sid, _nblk, valid) in enumerate(quad):
                for p0, p1, c0, c1 in _zero_regions(valid, bs):
                    mk[mid, p0:p1, h, c0:c1] = 0
    return mk


def _build_nc(windows, slot_blocks, dims, s_dt_name, p_dt_name, repeat):
    import hashlib
    import os

    import concourse.bass as bass
    import concourse.mybir as mybir
    import concourse.tile as tile
    from concourse import bacc

    ablate = set(os.environ.get("KERNEL_ABLATE", "").split(","))
    stp_bufs = int(os.environ.get("TUNE_STP", "4"))
    ptp_bufs = int(os.environ.get("TUNE_PTP", "14"))
    ovp_bufs = int(os.environ.get("TUNE_OVP", "2"))
    mark_reps = os.environ.get("MARK_REPS", "") == "1"

    # config fingerprint -> rtag length, so no two program variants share an
    # input signature (the neuron compile cache can alias same-signature HLO)
    cfg = repr((_KVER, sorted(ablate), stp_bufs, ptp_bufs, ovp_bufs,
                mark_reps, s_dt_name, p_dt_name, repeat, dims,
                [tuple(b) for b in slot_blocks]))
    cfg_h = int(hashlib.sha256(cfg.encode()).hexdigest(), 16) % 769

    S, L, HD, bs, qw, nch = dims
    s_dt = getattr(mybir.dt, s_dt_name)
    p_dt = getattr(mybir.dt, p_dt_name)
    f32 = mybir.dt.float32
    nq = qw * bs
    scale = float(HD) ** -0.5

    nwin = L // nq
    nc = bacc.Bacc("TRN2", debug=False)
    # dummy repeat-sized input: makes each repeat-variant's HLO structurally
    # unique so the neuron compile cache cannot alias them
    rtag_len = 16 * repeat + cfg_h
    rtag_d = nc.dram_tensor("rtag", [1, rtag_len], mybir.dt.float32,
                            kind="ExternalInput")
    qt_d = nc.dram_tensor("qt", [S, HD, L], s_dt, kind="ExternalInput")
    ktg_d = nc.dram_tensor("ktg", [S, HD, nch * 128], s_dt, kind="ExternalInput")
    vg_d = nc.dram_tensor("vg", [S, 128, nch, HD + 1], p_dt, kind="ExternalInput")
    # deduplicated 0/1 mask tiles for own-chunk pairs (possibly zero patterns)
    n_mk, mk_of = _mask_table(windows)
    mk_d = nc.dram_tensor("mk", [max(n_mk, 1), 128, QUAD_PACK, nq], p_dt,
                          kind="ExternalInput")
    # p-major output: out[s, p, w, d] = O[s, w*nq + p, d]; host untransposes
    out_d = nc.dram_tensor("out", [S, nq, nwin, HD], f32, kind="ExternalOutput")
    groups = _group_plan(windows)

    with tile.TileContext(nc) as tc:
        with (
            tc.tile_pool(name="big", bufs=2) as big,
            tc.tile_pool(name="ptp", bufs=ptp_bufs) as ptp,
            tc.tile_pool(name="onp", bufs=4) as onp,
            tc.tile_pool(name="stp", bufs=stp_bufs, space="PSUM") as stp,
            tc.tile_pool(name="ovp", bufs=ovp_bufs, space="PSUM") as ovp,
        ):
            rtag_t = big.tile([1, rtag_len], mybir.dt.float32, tag="rtag")
            nc.sync.dma_start(out=rtag_t, in_=rtag_d.ap())
            mk_t = big.tile([128, max(n_mk, 1), QUAD_PACK, nq], p_dt, tag="mk",
                            bufs=1)
            nc.sync.dma_start(
                out=mk_t, in_=mk_d.ap().rearrange("n p h q -> p n h q")
            )
            ngrp = len(groups)
            onedma = "onedma" in ablate  # probe: all input DMA before compute
            preload = []
            deferred_outs = []
            if onedma:
                for s in range(S):
                    qt_t = big.tile([HD, L], s_dt, tag="qt", bufs=S)
                    ktg_t = big.tile([HD, nch * 128], s_dt, tag="ktg", bufs=S)
                    vg_t = big.tile([128, nch, HD + 1], p_dt, tag="vg", bufs=S)
                    nc.sync.dma_start(out=qt_t, in_=qt_d.ap()[s])
                    nc.sync.dma_start(out=ktg_t, in_=ktg_d.ap()[s])
                    nc.sync.dma_start(out=vg_t, in_=vg_d.ap()[s])
                    preload.append((qt_t, ktg_t, vg_t))
            for _rep in range(repeat):
                for s in range(S):
                    if onedma:
                        qt_t, ktg_t, vg_t = preload[s]
                    else:
                        qt_t = big.tile([HD, L], s_dt, tag="qt")
                        ktg_t = big.tile([HD, nch * 128], s_dt, tag="ktg")
                        vg_t = big.tile([128, nch, HD + 1], p_dt, tag="vg")
                        # split the big input loads so (a) several DMA queues
                        # run in parallel and (b) group g's QK only waits on
                        # the pieces it reads (subtile deps)
                        for g in range(ngrp):
                            nc.sync.dma_start(
                                out=qt_t[:, g * 4 * nq : (g + 1) * 4 * nq],
                                in_=qt_d.ap()[s][
                                    :, g * 4 * nq : (g + 1) * 4 * nq
                                ],
                            )
                        ksplit = [
                            (a * 128, min(nch, a + max(1, nch // 8)) * 128)
                            for a in range(0, nch, max(1, nch // 8))
                        ]
                        for a, b in ksplit:
                            nc.sync.dma_start(
                                out=ktg_t[:, a:b], in_=ktg_d.ap()[s][:, a:b]
                            )
                        vsplit = [
                            (a, min(nch, a + max(1, nch // 4)))
                            for a in range(0, nch, max(1, nch // 4))
                        ]
                        for a, b in vsplit:
                            nc.sync.dma_start(
                                out=vg_t[:, a:b, :], in_=vg_d.ap()[s][:, a:b, :]
                            )
                    o_slice = big.tile([nq, nwin, HD], f32, tag="o_slice",
                                       bufs=S if onedma else None)
                    if onedma:
                        deferred_outs.append((s, o_slice))

                    def emit_qk(gi):
                        # QK + exp feed the ACT pipeline (the throughput
                        # floor): schedule them ahead of PV work so the
                        # in-order PE queue never starves ACT. offset keeps
                        # QK blocks ordered among themselves.
                        with tc.high_priority(offset=1 << 20):
                            return emit_qk_inner(gi)

                    def emit_qk_inner(gi):
                        shared, owns = groups[gi]
                        gw = len(owns)
                        gq = gw * nq
                        q0 = gi * 4 * nq
                        # own-chunk quads FIRST: their mms are cheap and were
                        # the sole source of ACT idle when emitted after the
                        # exp-paced shared pairs (trace: all ACT gaps waited
                        # on quad exps). Emitting them first lets the PE run
                        # them during the previous group's exp tail, so the
                        # ACT FIFO rolls pairs(g-1) -> quads(g) -> pairs(g)
                        # without starving.
                        own_pts = [[] for _ in owns]
                        for quad in _quads(owns):
                            nh = len(quad)
                            st_t = stp.tile([128, QUAD_PACK, nq], f32, tag="st", bufs=2)
                            pt_t = ptp.tile([128, QUAD_PACK, nq], p_dt, tag="pt")
                            for h, (m, sid, nblk, valid) in enumerate(quad):
                                nk = nblk * bs
                                if "skipst" not in ablate:
                                    nc.tensor.matmul(
                                        st_t[:nk, h, :],
                                        lhsT=ktg_t[
                                            :, sid * 128 : sid * 128 + nk
                                        ],
                                        rhs=qt_t[
                                            :,
                                            q0 + m * nq : q0 + (m + 1) * nq,
                                        ],
                                        start=True,
                                        stop=True,
                                    )
                            if "skipexp" not in ablate:
                                # full-tile exp; stale rows beyond each
                                # chunk's nk are never read downstream
                                nc.scalar.activation(
                                    pt_t[:, 0:nh, :],
                                    st_t[:, 0:nh, :],
                                    mybir.ActivationFunctionType.Exp,
                                    scale=scale,
                                )
                            mid = mk_of.get(_quad_sig(quad))
                            if mid is not None and "nomask" not in ablate:
                                nc.vector.tensor_mul(
                                    pt_t[:, 0:nh, :],
                                    pt_t[:, 0:nh, :],
                                    mk_t[:, mid, 0:nh, :],
                                )
                            for h, (m, sid, nblk, valid) in enumerate(quad):
                                own_pts[m].append((pt_t, sid, nblk * bs, h))
                        # group-shared chunks: S_T at width gq, exp over
                        # PAIRS of chunks (one exp spans 2 PSUM banks)
                        sh_pts = []
                        for i0 in range(0, len(shared), 2):
                            pr = shared[i0 : i0 + 2]
                            nh = len(pr)
                            st_t = stp.tile(
                                [128, 2, gq], f32, tag="stsh", bufs=2
                            )
                            pt_t = ptp.tile(
                                [128, 2, gq], p_dt, tag="ptsh", bufs=9
                            )
                            for h, (sid, nblk) in enumerate(pr):
                                nk = nblk * bs
                                if "skipst" not in ablate:
                                    nc.tensor.matmul(
                                        st_t[:nk, h, :],
                                        lhsT=ktg_t[
                                            :, sid * 128 : sid * 128 + nk
                                        ],
                                        rhs=qt_t[:, q0 : q0 + gq],
                                        start=True,
                                        stop=True,
                                    )
                            if "skipexp" not in ablate:
                                nc.scalar.activation(
                                    pt_t[:, 0:nh, :],
                                    st_t[:, 0:nh, :],
                                    mybir.ActivationFunctionType.Exp,
                                    scale=scale,
                                )
                            for h, (sid, nblk) in enumerate(pr):
                                sh_pts.append((pt_t, sid, nblk * bs, h))
                        return sh_pts, own_pts

                    def emit_pv(gi, sh_pts, own_pts):
                        shared, owns = groups[gi]
                        gw = len(owns)
                        # one PSUM bank holds all gw window accumulators
                        if "nopv" not in ablate:
                            ov_t = ovp.tile([128, gw, HD + 1], f32, tag="ov")
                            for m, wpts in enumerate(own_pts):
                                npv = len(sh_pts) + len(wpts)
                                ci = 0
                                for pt_t, sid, nk, h in sh_pts:
                                    nc.tensor.matmul(
                                        ov_t[:, m, :],
                                        lhsT=pt_t[:nk, h, m * nq : (m + 1) * nq],
                                        rhs=vg_t[:nk, sid, :],
                                        start=(ci == 0),
                                        stop=(ci == npv - 1),
                                    )
                                    ci += 1
                                for pt_t, sid, nk, h in wpts:
                                    nc.tensor.matmul(
                                        ov_t[:, m, :],
                                        lhsT=pt_t[:nk, h, :],
                                        rhs=vg_t[:nk, sid, :],
                                        start=(ci == 0),
                                        stop=(ci == npv - 1),
                                    )
                                    ci += 1
                        # normalize into the slice-wide output tile:
                        # one strided reciprocal + one broadcast multiply
                        # covers all gw windows of the group
                        if "nopv" in ablate:
                            nc.vector.memset(
                                o_slice[:, gi * 4 : gi * 4 + gw, :], 0.0
                            )
                        else:
                            rec_t = onp.tile([128, gw], f32, tag="rec")
                            if any(g for _o, g in owns):
                                den_t = onp.tile([128, gw], f32, tag="den")
                                nc.vector.tensor_scalar_max(
                                    den_t, ov_t[:, :, HD], 1e-37
                                )
                                nc.vector.reciprocal(rec_t, den_t)
                            else:
                                nc.vector.reciprocal(rec_t, ov_t[:, :, HD])
                            rec_b = bass.AP(
                                tensor=rec_t.tensor,
                                offset=rec_t.offset,
                                ap=list(rec_t.ap) + [[0, HD]],
                            )
                            nc.vector.tensor_mul(
                                o_slice[:, gi * 4 : gi * 4 + gw, :],
                                ov_t[:, :, 0:HD],
                                rec_b,
                            )
                        if mark_reps:
                            # on DVE, not ScalarE: the strict ACT FIFO would
                            # head-block pending exps on this chain
                            nc.vector.tensor_scalar_mul(
                                o_slice[:, gi * 4 : gi * 4 + gw, :],
                                o_slice[:, gi * 4 : gi * 4 + gw, :],
                                float(_rep + 1),
                            )
                        if not onedma:
                            # stream this group's output rows out as soon as
                            # they are normalized: overlaps the tail DMA
                            nc.sync.dma_start(
                                out=out_d.ap()[s][:, gi * 4 : gi * 4 + gw, :],
                                in_=o_slice[:, gi * 4 : gi * 4 + gw, :],
                            )

                    # software pipeline: issue QK(g+1) before PV(g) so the
                    # in-order PE queue never idles on exp/mask of group g
                    pending = None
                    for gi in range(ngrp):
                        pts = emit_qk(gi)
                        if pending is not None:
                            emit_pv(pending[0], pending[1], pending[2])
                        pending = (gi, pts[0], pts[1])
                    emit_pv(pending[0], pending[1], pending[2])
            for s_, osl_ in deferred_outs:
                nc.sync.dma_start(out=out_d.ap()[s_], in_=osl_)
    nc.compile()
    return nc


def _np_dt(name):
    if name == "float32":
        return np.float32
    if name == "float16":
        return np.float16
    import ml_dtypes

    return np.dtype(getattr(ml_dtypes, name))


def _prepare(query, key, value, rows, cols, block, repeat):
    B, H, L, HD = query.shape
    bs = int(block)
    nb = L // bs
    qw = max(1, 128 // bs)
    cache_key = (
        query.shape,
        bs,
        rows.tobytes(),
        cols.tobytes(),
        S_DT_NAME,
        P_DT_NAME,
        repeat,
    )
    if cache_key in _cache:
        return _cache[cache_key]

    windows, slot_blocks = _build_plan(np.asarray(rows), np.asarray(cols), nb, qw)
    nch = len(slot_blocks)
    dims = (B * H // N_CORES, L, HD, bs, qw, nch)
    nc = _build_nc(windows, slot_blocks, dims, S_DT_NAME, P_DT_NAME, repeat)
    _cache[cache_key] = (nc, windows, slot_blocks, dims)
    return _cache[cache_key]


def _host_inputs(nc, windows, slot_blocks, dims, query, key, value):
    """Pre-gather host-side inputs -> per-core in_maps (repeat-independent)."""
    S, L, HD, bs, qw, nch = dims
    B, H = query.shape[0], query.shape[1]
    BH = B * H
    s_np = _np_dt(S_DT_NAME)
    p_np = _np_dt(P_DT_NAME)

    q2 = query.reshape(BH, L, HD)
    k2 = key.reshape(BH, L, HD)
    v2 = value.reshape(BH, L, HD)
    qt = np.ascontiguousarray(q2.transpose(0, 2, 1)).astype(s_np)
    ktg = np.zeros((BH, HD, nch, 128), s_np)
    vg = np.zeros((BH, 128, nch, HD + 1), p_np)
    for sid, blocks in enumerate(slot_blocks):
        for kb, c in enumerate(blocks):
            kblk = k2[:, c * bs : (c + 1) * bs, :]
            ktg[:, :, sid, kb * bs : (kb + 1) * bs] = kblk.transpose(0, 2, 1)
            vg[:, kb * bs : (kb + 1) * bs, sid, :HD] = v2[:, c * bs : (c + 1) * bs, :]
            vg[:, kb * bs : (kb + 1) * bs, sid, HD] = 1.0
    ktg = ktg.reshape(BH, HD, nch * 128)

    rtag_len = None
    for alloc in nc.m.functions[0].allocations:
        if getattr(alloc, "memorylocations", None) and \
                alloc.memorylocations[0].name == "rtag":
            rtag_len = alloc.tensor_shape[1]
    rtag = np.zeros((1, rtag_len), np.float32)
    mk = _build_masks(windows, dims, p_np)
    in_maps = []
    for c in range(N_CORES):
        sl = slice(c * S, (c + 1) * S)
        in_maps.append({"qt": qt[sl], "ktg": ktg[sl], "vg": vg[sl],
                        "rtag": rtag, "mk": mk})
    return in_maps


def _unshard(out, B, H, L, HD):
    """[8, S, nq, nwin, HD] p-major core outputs -> [B, H, L, HD]."""
    BH = B * H
    nq = out.shape[2]
    out = out.reshape(BH, nq, L // nq, HD).transpose(0, 2, 1, 3)
    return np.ascontiguousarray(out.reshape(B, H, L, HD)).astype(np.float32)


def kernel(query, key, value, rows, cols, block):
    from concourse import bass_utils

    query = np.asarray(query)
    key = np.asarray(key)
    value = np.asarray(value)
    rows = np.asarray(rows)
    cols = np.asarray(cols)

    nc, windows, slot_blocks, dims = _prepare(
        query, key, value, rows, cols, block, repeat=1
    )
    S, L, HD, bs, qw, nch = dims
    B, H = query.shape[0], query.shape[1]
    in_maps = _host_inputs(nc, windows, slot_blocks, dims, query, key, value)

    # the axon-tunneled devices occasionally throw a transient
    # NRT_EXEC_UNIT_UNRECOVERABLE on first contact; retry before giving up
    last_err = None
    for attempt in range(3):
        try:
            res = bass_utils.run_bass_kernel_spmd(
                nc, in_maps, core_ids=list(range(N_CORES))
            )
            break
        except Exception as e:  # noqa: BLE001
            last_err = e
            import time as _time

            _time.sleep(2.0 * (attempt + 1))
    else:
        raise last_err
    out = np.stack([res.results[c]["out"] for c in range(N_CORES)])
    return _unshard(out, B, H, L, HD)



# revision 21
# speedup vs baseline: 1.0229x; 1.0229x over previous
"""Block-sparse (DeepSpeed fixed-layout) self-attention on 8 Trainium2 cores.

Strategy
--------
Shard the 32 (batch, head) slices across 8 cores (4 each, pure data parallel).
For each slice, queries are processed in windows of 128 rows (4 key-blocks of
32). The union of active key blocks for a window is split into "chunk slots"
of up to 128 keys; slots are deduplicated across windows (the fixed layout's
global stripe makes most slots shared). Host-side numpy pre-gathers:
  QT  [S, 64, L]            query transposed (hd on partitions)
  KTg [S, 64, nch*128]      gathered+transposed key chunks
  Vg  [S, 128, nch, 65]     gathered value chunks with a ones column
On device, per window and chunk:
  S_T  = KT_chunk.T-matmul (scores arrive keys-on-partitions: no P transpose)
  P    = exp(scale * S_T)   on ACT, straight from PSUM
  mask: memset invalid (key-block, query-block) cells to zero
  O~  += P.T @ [V | 1]      accumulated in PSUM; col 64 = softmax denominator
then O = O~[:, :64] * (1 / O~[:, 64]) and DMA out. exp() needs no max
subtraction: scores are ~N(0,1) after scaling, far from fp32 overflow.
"""

import sys

sys.path.insert(0, "/opt/trn_rl_repo")

import numpy as np

N_CORES = 8
_KVER = "v16"  # bump on any codegen change: feeds the rtag config fingerprint

# dtype knobs: storage+matmul dtype for scores (QT/KTg) and probs (P/Vg)
S_DT_NAME = "bfloat16"
P_DT_NAME = "bfloat16"

_cache = {}


def _build_plan(rows, cols, nb, qw):
    """Per query-window chunk lists + deduplicated key-chunk slots.

    Returns (windows, slot_blocks):
      windows: list (one per window) of chunks (slot_id, n_blocks, valid)
               where valid[kb, j] says whether key-block kb of the chunk is
               attended by query-block j of the window.
      slot_blocks: slot_id -> list of key block ids stored in that slot.
    """
    from collections import Counter

    row_cnt = [Counter() for _ in range(nb)]
    for r, c in zip(rows.tolist(), cols.tolist()):
        row_cnt[int(r)][int(c)] += 1

    slots = {}
    slot_blocks = []
    windows = []
    for w0 in range(0, nb, qw):
        cnts = [row_cnt[w0 + j] for j in range(qw)]
        cols_set = sorted(set().union(*[set(c.keys()) for c in cnts]))
        entries = []  # (block, occurrence)
        for c in cols_set:
            m = max(cnt[c] for cnt in cnts)
            entries.extend((c, k) for k in range(m))
        # maximal equal-stride runs -> chunk boundaries shared across windows
        runs = []
        i, n = 0, len(entries)
        while i < n:
            if i + 1 < n:
                stride = entries[i + 1][0] - entries[i][0]
                j = i + 1
                while j + 1 < n and entries[j + 1][0] - entries[j][0] == stride:
                    j += 1
            else:
                stride, j = 1, i
            runs.append((i, j + 1, stride))
            i = j + 1
        wchunks = []
        for a, b, stride in runs:
            for t in range(a, b, qw):
                grp = entries[t : min(t + qw, b)]
                start = grp[0][0]
                key = (start, stride if len(grp) > 1 else 1)
                sid = slots.get(key)
                if sid is None:
                    sid = len(slot_blocks)
                    slots[key] = sid
                    slot_blocks.append([])
                if len(slot_blocks[sid]) < len(grp):
                    slot_blocks[sid] = [start + key[1] * u for u in range(len(grp))]
                valid = np.ones((len(grp), qw), dtype=bool)
                for kb, (c, k) in enumerate(grp):
                    for j in range(qw):
                        valid[kb, j] = k < cnts[j][c]
                wchunks.append((sid, len(grp), valid))
        # does any query row of this window have no valid key at all?
        anyvalid = np.zeros(qw, dtype=bool)
        for _sid, _n, valid in wchunks:
            anyvalid |= valid.any(axis=0)
        windows.append((wchunks, not anyvalid.all()))
    return windows, slot_blocks


def _zero_regions(valid, bs):
    """Invalid (key-block, query-block) cells as memset rectangles."""
    regs = []
    nkb, qw = valid.shape
    for kb in range(nkb):
        j = 0
        while j < qw:
            if not valid[kb, j]:
                j0 = j
                while j < qw and not valid[kb, j]:
                    j += 1
                regs.append((kb * bs, (kb + 1) * bs, j0 * bs, j * bs))
            else:
                j += 1
    return regs


def _group_plan(windows, qw_groups=4):
    """Group consecutive windows; extract chunks shared (all-valid) by every
    window of the group so their S_T/exp run once at group width."""
    groups = []
    for g0 in range(0, len(windows), qw_groups):
        ws = windows[g0 : g0 + qw_groups]
        sets = [
            {(sid, nblk) for sid, nblk, valid in w if valid.all()}
            for w, _g in ws
        ]
        shared = sorted(set.intersection(*sets)) if len(ws) == qw_groups else []
        shared_set = set(shared)
        owns = []
        for w, guard in ws:
            owns.append(
                (
                    [
                        (sid, nblk, valid)
                        for sid, nblk, valid in w
                        if (sid, nblk) not in shared_set or not valid.all()
                    ],
                    guard,
                )
            )
        groups.append((shared, owns))
    return groups


QUAD_PACK = 4


def _quads(owns):
    """Flatten a group's per-window own chunks, QUAD_PACK halves per tile."""
    halves = []
    for m, (own, _guard) in enumerate(owns):
        for sid, nblk, valid in own:
            halves.append((m, sid, nblk, valid))
    return [halves[i : i + QUAD_PACK] for i in range(0, len(halves), QUAD_PACK)]


def _quad_sig(quad):
    # mask pattern depends only on each half's (nblk, validity) - not sid
    return tuple((nblk, valid.tobytes()) for _m, _sid, nblk, valid in quad)


def _mask_table(windows, qw_groups=4):
    """Dedup mask patterns over own-chunk quads. Returns (n_masks, map from
    quad-signature -> mask id); quads with no zero region map to nothing."""
    mk_of = {}
    n = 0
    for shared, owns in _group_plan(windows, qw_groups):
        for quad in _quads(owns):
            sig = _quad_sig(quad)
            if sig in mk_of:
                continue
            if any(not valid.all() for _m, _sid, _nblk, valid in quad):
                mk_of[sig] = n
                n += 1
    return n, mk_of


def _build_masks(windows, dims, p_np, qw_groups=4):
    """Materialize the deduplicated mask tiles: [n_mk, 128, 4, nq]."""
    S, L, HD, bs, qw, nch = dims
    nq = qw * bs
    n_mk, mk_of = _mask_table(windows, qw_groups)
    mk = np.ones((max(n_mk, 1), 128, QUAD_PACK, nq), p_np)
    done = set()
    for shared, owns in _group_plan(windows, qw_groups):
        for quad in _quads(owns):
            mid = mk_of.get(_quad_sig(quad))
            if mid is None or mid in done:
                continue
            done.add(mid)
            for h, (_m, _sid, _nblk, valid) in enumerate(quad):
                for p0, p1, c0, c1 in _zero_regions(valid, bs):
                    mk[mid, p0:p1, h, c0:c1] = 0
    return mk


def _build_nc(windows, slot_blocks, dims, s_dt_name, p_dt_name, repeat):
    import hashlib
    import os

    import concourse.bass as bass
    import concourse.mybir as mybir
    import concourse.tile as tile
    from concourse import bacc

    ablate = set(os.environ.get("KERNEL_ABLATE", "").split(","))
    stp_bufs = int(os.environ.get("TUNE_STP", "4"))
    ptp_bufs = int(os.environ.get("TUNE_PTP", "14"))
    ovp_bufs = int(os.environ.get("TUNE_OVP", "1"))
    mark_reps = os.environ.get("MARK_REPS", "") == "1"

    # config fingerprint -> rtag length, so no two program variants share an
    # input signature (the neuron compile cache can alias same-signature HLO)
    cfg = repr((_KVER, sorted(ablate), stp_bufs, ptp_bufs, ovp_bufs,
                mark_reps, s_dt_name, p_dt_name, repeat, dims,
                [tuple(b) for b in slot_blocks]))
    cfg_h = int(hashlib.sha256(cfg.encode()).hexdigest(), 16) % 769

    S, L, HD, bs, qw, nch = dims
    s_dt = getattr(mybir.dt, s_dt_name)
    p_dt = getattr(mybir.dt, p_dt_name)
    f32 = mybir.dt.float32
    nq = qw * bs
    scale = float(HD) ** -0.5

    nwin = L // nq
    nc = bacc.Bacc("TRN2", debug=False)
    # dummy repeat-sized input: makes each repeat-variant's HLO structurally
    # unique so the neuron compile cache cannot alias them
    rtag_len = 16 * repeat + cfg_h
    rtag_d = nc.dram_tensor("rtag", [1, rtag_len], mybir.dt.float32,
                            kind="ExternalInput")
    qt_d = nc.dram_tensor("qt", [S, HD, L], s_dt, kind="ExternalInput")
    ktg_d = nc.dram_tensor("ktg", [S, HD, nch * 128], s_dt, kind="ExternalInput")
    vg_d = nc.dram_tensor("vg", [S, 128, nch, HD + 1], p_dt, kind="ExternalInput")
    # deduplicated 0/1 mask tiles for own-chunk pairs (possibly zero patterns)
    n_mk, mk_of = _mask_table(windows)
    mk_d = nc.dram_tensor("mk", [max(n_mk, 1), 128, QUAD_PACK, nq], p_dt,
                          kind="ExternalInput")
    # p-major output: out[s, p, w, d] = O[s, w*nq + p, d]; host untransposes
    out_d = nc.dram_tensor("out", [S, nq, nwin, HD], f32, kind="ExternalOutput")
    groups = _group_plan(windows)

    with tile.TileContext(nc) as tc:
        with (
            tc.tile_pool(name="big", bufs=2) as big,
            tc.tile_pool(name="ptp", bufs=ptp_bufs) as ptp,
            tc.tile_pool(name="onp", bufs=4) as onp,
            tc.tile_pool(name="stp", bufs=stp_bufs, space="PSUM") as stp,
            tc.tile_pool(name="ovp", bufs=ovp_bufs, space="PSUM") as ovp,
        ):
            rtag_t = big.tile([1, rtag_len], mybir.dt.float32, tag="rtag")
            nc.sync.dma_start(out=rtag_t, in_=rtag_d.ap())
            mk_t = big.tile([128, max(n_mk, 1), QUAD_PACK, nq], p_dt, tag="mk",
                            bufs=1)
            nc.sync.dma_start(
                out=mk_t, in_=mk_d.ap().rearrange("n p h q -> p n h q")
            )
            ngrp = len(groups)
            onedma = "onedma" in ablate  # probe: all input DMA before compute
            preload = []
            deferred_outs = []
            if onedma:
                for s in range(S):
                    qt_t = big.tile([HD, L], s_dt, tag="qt", bufs=S)
                    ktg_t = big.tile([HD, nch * 128], s_dt, tag="ktg", bufs=S)
                    vg_t = big.tile([128, nch, HD + 1], p_dt, tag="vg", bufs=S)
                    nc.sync.dma_start(out=qt_t, in_=qt_d.ap()[s])
                    nc.sync.dma_start(out=ktg_t, in_=ktg_d.ap()[s])
                    nc.sync.dma_start(out=vg_t, in_=vg_d.ap()[s])
                    preload.append((qt_t, ktg_t, vg_t))
            for _rep in range(repeat):
                for s in range(S):
                    if onedma:
                        qt_t, ktg_t, vg_t = preload[s]
                    else:
                        qt_t = big.tile([HD, L], s_dt, tag="qt")
                        ktg_t = big.tile([HD, nch * 128], s_dt, tag="ktg")
                        vg_t = big.tile([128, nch, HD + 1], p_dt, tag="vg")
                        # split the big input loads so (a) several DMA queues
                        # run in parallel and (b) group g's QK only waits on
                        # the pieces it reads (subtile deps)
                        for g in range(ngrp):
                            nc.sync.dma_start(
                                out=qt_t[:, g * 4 * nq : (g + 1) * 4 * nq],
                                in_=qt_d.ap()[s][
                                    :, g * 4 * nq : (g + 1) * 4 * nq
                                ],
                            )
                        ksplit = [
                            (a * 128, min(nch, a + max(1, nch // 8)) * 128)
                            for a in range(0, nch, max(1, nch // 8))
                        ]
                        for a, b in ksplit:
                            nc.sync.dma_start(
                                out=ktg_t[:, a:b], in_=ktg_d.ap()[s][:, a:b]
                            )
                        vsplit = [
                            (a, min(nch, a + max(1, nch // 4)))
                            for a in range(0, nch, max(1, nch // 4))
                        ]
                        for a, b in vsplit:
                            nc.sync.dma_start(
                                out=vg_t[:, a:b, :], in_=vg_d.ap()[s][:, a:b, :]
                            )
                    o_slice = big.tile([nq, nwin, HD], f32, tag="o_slice",
                                       bufs=S if onedma else None)
                    if onedma:
                        deferred_outs.append((s, o_slice))

                    def emit_qk(gi):
                        # QK + exp feed the ACT pipeline (the throughput
                        # floor): schedule them ahead of PV work so the
                        # in-order PE queue never starves ACT. offset keeps
                        # QK blocks ordered among themselves.
                        with tc.high_priority(offset=1 << 20):
                            return emit_qk_inner(gi)

                    def emit_qk_inner(gi):
                        shared, owns = groups[gi]
                        gw = len(owns)
                        gq = gw * nq
                        q0 = gi * 4 * nq
                        # group-shared chunks: S_T at width gq, exp over
                        # PAIRS of chunks (one exp spans 2 PSUM banks)
                        sh_pts = []
                        for i0 in range(0, len(shared), 2):
                            pr = shared[i0 : i0 + 2]
                            nh = len(pr)
                            st_t = stp.tile(
                                [128, 2, gq], f32, tag="stsh", bufs=3
                            )
                            pt_t = ptp.tile(
                                [128, 2, gq], p_dt, tag="ptsh", bufs=9
                            )
                            for h, (sid, nblk) in enumerate(pr):
                                nk = nblk * bs
                                if "skipst" not in ablate:
                                    nc.tensor.matmul(
                                        st_t[:nk, h, :],
                                        lhsT=ktg_t[
                                            :, sid * 128 : sid * 128 + nk
                                        ],
                                        rhs=qt_t[:, q0 : q0 + gq],
                                        start=True,
                                        stop=True,
                                    )
                            if "skipexp" not in ablate:
                                nc.scalar.activation(
                                    pt_t[:, 0:nh, :],
                                    st_t[:, 0:nh, :],
                                    mybir.ActivationFunctionType.Exp,
                                    scale=scale,
                                )
                            for h, (sid, nblk) in enumerate(pr):
                                sh_pts.append((pt_t, sid, nblk * bs, h))
                        # per-window extra chunks (masked/partial):
                        # QUAD_PACK halves per PSUM tile, one exp + one
                        # mask-mul per quad
                        own_pts = [[] for _ in owns]
                        for quad in _quads(owns):
                            nh = len(quad)
                            st_t = stp.tile([128, QUAD_PACK, nq], f32, tag="st", bufs=1)
                            pt_t = ptp.tile([128, QUAD_PACK, nq], p_dt, tag="pt")
                            for h, (m, sid, nblk, valid) in enumerate(quad):
                                nk = nblk * bs
                                if "skipst" not in ablate:
                                    nc.tensor.matmul(
                                        st_t[:nk, h, :],
                                        lhsT=ktg_t[
                                            :, sid * 128 : sid * 128 + nk
                                        ],
                                        rhs=qt_t[
                                            :,
                                            q0 + m * nq : q0 + (m + 1) * nq,
                                        ],
                                        start=True,
                                        stop=True,
                                    )
                            if "skipexp" not in ablate:
                                # full-tile exp; stale rows beyond each
                                # chunk's nk are never read downstream
                                nc.scalar.activation(
                                    pt_t[:, 0:nh, :],
                                    st_t[:, 0:nh, :],
                                    mybir.ActivationFunctionType.Exp,
                                    scale=scale,
                                )
                            mid = mk_of.get(_quad_sig(quad))
                            if mid is not None and "nomask" not in ablate:
                                nc.vector.tensor_mul(
                                    pt_t[:, 0:nh, :],
                                    pt_t[:, 0:nh, :],
                                    mk_t[:, mid, 0:nh, :],
                                )
                            for h, (m, sid, nblk, valid) in enumerate(quad):
                                own_pts[m].append((pt_t, sid, nblk * bs, h))
                        return sh_pts, own_pts

                    def emit_pv(gi, sh_pts, own_pts):
                        shared, owns = groups[gi]
                        gw = len(owns)
                        # one PSUM bank holds all gw window accumulators
                        if "nopv" not in ablate:
                            ov_t = ovp.tile([128, gw, HD + 1], f32, tag="ov")
                            for m, wpts in enumerate(own_pts):
                                npv = len(sh_pts) + len(wpts)
                                ci = 0
                                for pt_t, sid, nk, h in sh_pts:
                                    nc.tensor.matmul(
                                        ov_t[:, m, :],
                                        lhsT=pt_t[:nk, h, m * nq : (m + 1) * nq],
                                        rhs=vg_t[:nk, sid, :],
                                        start=(ci == 0),
                                        stop=(ci == npv - 1),
                                    )
                                    ci += 1
                                for pt_t, sid, nk, h in wpts:
                                    nc.tensor.matmul(
                                        ov_t[:, m, :],
                                        lhsT=pt_t[:nk, h, :],
                                        rhs=vg_t[:nk, sid, :],
                                        start=(ci == 0),
                                        stop=(ci == npv - 1),
                                    )
                                    ci += 1
                        # normalize into the slice-wide output tile:
                        # one strided reciprocal + one broadcast multiply
                        # covers all gw windows of the group
                        if "nopv" in ablate:
                            nc.vector.memset(
                                o_slice[:, gi * 4 : gi * 4 + gw, :], 0.0
                            )
                        else:
                            rec_t = onp.tile([128, gw], f32, tag="rec")
                            if any(g for _o, g in owns):
                                den_t = onp.tile([128, gw], f32, tag="den")
                                nc.vector.tensor_scalar_max(
                                    den_t, ov_t[:, :, HD], 1e-37
                                )
                                nc.vector.reciprocal(rec_t, den_t)
                            else:
                                nc.vector.reciprocal(rec_t, ov_t[:, :, HD])
                            rec_b = bass.AP(
                                tensor=rec_t.tensor,
                                offset=rec_t.offset,
                                ap=list(rec_t.ap) + [[0, HD]],
                            )
                            nc.vector.tensor_mul(
                                o_slice[:, gi * 4 : gi * 4 + gw, :],
                                ov_t[:, :, 0:HD],
                                rec_b,
                            )
                        if mark_reps:
                            # on DVE, not ScalarE: the strict ACT FIFO would
                            # head-block pending exps on this chain
                            nc.vector.tensor_scalar_mul(
                                o_slice[:, gi * 4 : gi * 4 + gw, :],
                                o_slice[:, gi * 4 : gi * 4 + gw, :],
                                float(_rep + 1),
                            )
                        if not onedma:
                            # stream this group's output rows out as soon as
                            # they are normalized: overlaps the tail DMA
                            nc.sync.dma_start(
                                out=out_d.ap()[s][:, gi * 4 : gi * 4 + gw, :],
                                in_=o_slice[:, gi * 4 : gi * 4 + gw, :],
                            )

                    # software pipeline: issue QK(g+1) before PV(g) so the
                    # in-order PE queue never idles on exp/mask of group g
                    pending = None
                    for gi in range(ngrp):
                        pts = emit_qk(gi)
                        if pending is not None:
                            emit_pv(pending[0], pending[1], pending[2])
                        pending = (gi, pts[0], pts[1])
                    emit_pv(pending[0], pending[1], pending[2])
            for s_, osl_ in deferred_outs:
                nc.sync.dma_start(out=out_d.ap()[s_], in_=osl_)
    nc.compile()
    return nc


def _np_dt(name):
    if name == "float32":
        return np.float32
    if name == "float16":
        return np.float16
    import ml_dtypes

    return np.dtype(getattr(ml_dtypes, name))


def _prepare(query, key, value, rows, cols, block, repeat):
    B, H, L, HD = query.shape
    bs = int(block)
    nb = L // bs
    qw = max(1, 128 // bs)
    cache_key = (
        query.shape,
        bs,
        rows.tobytes(),
        cols.tobytes(),
        S_DT_NAME,
        P_DT_NAME,
        repeat,
    )
    if cache_key in _cache:
        return _cache[cache_key]

    windows, slot_blocks = _build_plan(np.asarray(rows), np.asarray(cols), nb, qw)
    nch = len(slot_blocks)
    dims = (B * H // N_CORES, L, HD, bs, qw, nch)
    nc = _build_nc(windows, slot_blocks, dims, S_DT_NAME, P_DT_NAME, repeat)
    _cache[cache_key] = (nc, windows, slot_blocks, dims)
    return _cache[cache_key]


def _host_inputs(nc, windows, slot_blocks, dims, query, key, value):
    """Pre-gather host-side inputs -> per-core in_maps (repeat-independent)."""
    S, L, HD, bs, qw, nch = dims
    B, H = query.shape[0], query.shape[1]
    BH = B * H
    s_np = _np_dt(S_DT_NAME)
    p_np = _np_dt(P_DT_NAME)

    q2 = query.reshape(BH, L, HD)
    k2 = key.reshape(BH, L, HD)
    v2 = value.reshape(BH, L, HD)
    qt = np.ascontiguousarray(q2.transpose(0, 2, 1)).astype(s_np)
    ktg = np.zeros((BH, HD, nch, 128), s_np)
    vg = np.zeros((BH, 128, nch, HD + 1), p_np)
    for sid, blocks in enumerate(slot_blocks):
        for kb, c in enumerate(blocks):
            kblk = k2[:, c * bs : (c + 1) * bs, :]
            ktg[:, :, sid, kb * bs : (kb + 1) * bs] = kblk.transpose(0, 2, 1)
            vg[:, kb * bs : (kb + 1) * bs, sid, :HD] = v2[:, c * bs : (c + 1) * bs, :]
            vg[:, kb * bs : (kb + 1) * bs, sid, HD] = 1.0
    ktg = ktg.reshape(BH, HD, nch * 128)

    rtag_len = None
    for alloc in nc.m.functions[0].allocations:
        if getattr(alloc, "memorylocations", None) and \
                alloc.memorylocations[0].name == "rtag":
            rtag_len = alloc.tensor_shape[1]
    rtag = np.zeros((1, rtag_len), np.float32)
    mk = _build_masks(windows, dims, p_np)
    in_maps = []
    for c in range(N_CORES):
        sl = slice(c * S, (c + 1) * S)
        in_maps.append({"qt": qt[sl], "ktg": ktg[sl], "vg": vg[sl],
                        "rtag": rtag, "mk": mk})
    return in_maps


def _unshard(out, B, H, L, HD):
    """[8, S, nq, nwin, HD] p-major core outputs -> [B, H, L, HD]."""
    BH = B * H
    nq = out.shape[2]
    out = out.reshape(BH, nq, L // nq, HD).transpose(0, 2, 1, 3)
    return np.ascontiguousarray(out.reshape(B, H, L, HD)).astype(np.float32)


def kernel(query, key, value, rows, cols, block):
    from concourse import bass_utils

    query = np.asarray(query)
    key = np.asarray(key)
    value = np.asarray(value)
    rows = np.asarray(rows)
    cols = np.asarray(cols)

    nc, windows, slot_blocks, dims = _prepare(
        query, key, value, rows, cols, block, repeat=1
    )
    S, L, HD, bs, qw, nch = dims
    B, H = query.shape[0], query.shape[1]
    in_maps = _host_inputs(nc, windows, slot_blocks, dims, query, key, value)

    # the axon-tunneled devices occasionally throw a transient
    # NRT_EXEC_UNIT_UNRECOVERABLE on first contact; retry before giving up
    last_err = None
    for attempt in range(3):
        try:
            res = bass_utils.run_bass_kernel_spmd(
                nc, in_maps, core_ids=list(range(N_CORES))
            )
            break
        except Exception as e:  # noqa: BLE001
            last_err = e
            import time as _time

            _time.sleep(2.0 * (attempt + 1))
    else:
        raise last_err
    out = np.stack([res.results[c]["out"] for c in range(N_CORES)])
    return _unshard(out, B, H, L, HD)

